# revision 21
# baseline (speedup 1.0000x reference)
"""Trainium2 Bass kernel: scaled-softmax attention, B=4 H=16 S=2048 D=64.

Sharding: batch*heads (64) across 8 NeuronCores, 8 heads per core.

Dual-engine softmax exponentials + fp16 QK operands, fully PE-bound.

Per head, streaming over (query-half, k-block) chunks of [128 keys x 1024 q]:
  S'[kb] = kT_aug[kb] @ qT_aug      (fp16 matmuls, K=65: 64 dims + a row that
            adds  C - m_hat_q  per query, C = (16256-sigma)/alpha)
  P[kb]:
    cols 0:ACT_W    (ACT): exp(S' - C) -> bf16          (true exp)
    cols ACT_W:1024 (DVE): int16(max(alpha*S', 0))      (Schraudolph)
        whose int16 bits ARE bf16(2^(alpha*(s-m_hat)/128)) ~= exp(s-m_hat);
        host pre-sorts queries by m_hat so the low-error ACT path gets the
        peaked rows where approximation error averages out worst.
  av[qhalf] += [v|1][kb] @ P[kb]    (fp16 x bf16, K=128, PSUM accumulate;
            ones-column makes row 64 the softmax denominator)
  AV is software-pipelined three chunks behind QK so neither the exp engines'
  latency nor the drain sits on the PE critical path; PSUM: 3x2 banks QK
  double-buffer ring + 2 banks av accumulator.

Host does marshaling: q scaled by 1/(scale_factor*inv_scale), m_hat =
5*||q_scaled||, per-head query permutation (peaked -> ACT columns), fp16
rounding, transposes; and on the way out the per-query divide by the
denominator row, inverse permutation, transpose.
"""

import os
import sys

sys.path.insert(0, "/opt/trn_rl_repo")

from contextlib import ExitStack

import numpy as np

import concourse.bass as bass
import concourse.tile as tile
from concourse import bacc, mybir
from concourse.bass_utils import run_bass_kernel_spmd
from concourse.masks import make_identity

B, H, S, D = 4, 16, 2048, 64
N_CORES = 8
HPC = (B * H) // N_CORES  # heads per core
KB = S // 128  # 16 k-blocks
DA = D + 1  # augmented contraction dim (65)
NQ = 1024  # query chunk (2 PSUM banks)
ACT_W = 704  # ACT-path (true exp) queries per chunk
ALPHA = 128.0 / float(np.log(2.0))  # 184.664965...
SIGMA = 5.52  # Schraudolph bias correction (centers the sawtooth error)
CBIAS = (16256.0 - SIGMA) / ALPHA  # ~88.0 ; folded into the aug row

F32 = mybir.dt.float32
BF16 = mybir.dt.bfloat16
F16 = mybir.dt.float16
I16 = mybir.dt.int16

LAST_RESULT = None
_CACHED_NC = None


def _maybe_install_ntff_hook():
    """BASS_TRACE=1 needs antenv.axon_hooks, absent from this image; inject it."""
    if not os.environ.get("BASS_TRACE") or "antenv.axon_hooks" in sys.modules:
        return
    try:
        import types

        import antenv
        from trn_agent_boot.trn_boot import _ntff_profile_via_ctypes

        mod = types.ModuleType("antenv.axon_hooks")
        mod._hook = None
        mod.set_axon_ntff_profile_hook = lambda h: setattr(mod, "_hook", h)
        mod.get_axon_ntff_profile_hook = lambda: mod._hook
        sys.modules["antenv.axon_hooks"] = mod
        antenv.axon_hooks = mod
        mod.set_axon_ntff_profile_hook(
            _ntff_profile_via_ctypes("/opt/axon/libaxon_pjrt.so")
        )
    except Exception:
        os.environ["BASS_NEVER_TRACE"] = "1"


def _build_nc():
    nc = bacc.Bacc("TRN2", target_bir_lowering=False, debug=False)

    d_qT = nc.dram_tensor("qT", [HPC, DA, S], F16, kind="ExternalInput").ap()
    d_kT = nc.dram_tensor("kT", [HPC, DA, S], F16, kind="ExternalInput").ap()
    d_v = nc.dram_tensor("v", [HPC, 128, KB, DA], F16, kind="ExternalInput").ap()
    d_out = nc.dram_tensor("outT", [HPC, DA, S], F32, kind="ExternalOutput").ap()

    with tile.TileContext(nc) as tc, ExitStack() as ctx:
        cpool = ctx.enter_context(tc.tile_pool(name="consts", bufs=1))
        inpool = ctx.enter_context(tc.tile_pool(name="in", bufs=3))
        ptpool = ctx.enter_context(tc.tile_pool(name="pt", bufs=6))
        wkpool = ctx.enter_context(tc.tile_pool(name="wk", bufs=2))
        qkp = ctx.enter_context(tc.tile_pool(name="qkp", bufs=3, space="PSUM"))
        avp = ctx.enter_context(tc.tile_pool(name="avp", bufs=1, space="PSUM"))

        ident = cpool.tile([DA, DA], F32)
        make_identity(nc, ident[:])
        t_bias = cpool.tile([128, 1], F32)
        nc.vector.memset(t_bias[:], -CBIAS)
        t_warm = cpool.tile([1, 1], F32)
        # trigger the ACT exp table load while input DMAs run
        nc.scalar.activation(
            t_warm[:], ident[0:1, 0:1], mybir.ActivationFunctionType.Exp
        )
        # software pipeline: AV of chunk i-2 is emitted while QK of chunk i
        # runs, so the exp engines' latency is hidden behind the PE stream.
        from collections import deque

        pend = deque()  # (pt_tile, t_v, t_av, kb, outs, h, half)

        def emit_av(p):
            pt, t_v, t_av, kb, outs, h, half = p
            for j in range(2):
                nc.tensor.matmul(
                    t_av[:, j * 512 : (j + 1) * 512],
                    t_v[:, kb, :],
                    pt[:, j * 512 : (j + 1) * 512],
                    start=(kb == 0),
                    stop=(kb == KB - 1),
                )
            if kb == KB - 1:
                # drain: split the PSUM->SBUF copy across ACT and DVE
                nc.scalar.activation(
                    outs[:, 0:512],
                    t_av[:, 0:512],
                    mybir.ActivationFunctionType.Copy,
                )
                nc.vector.tensor_copy(outs[:, 512:NQ], t_av[:, 512:NQ])
                q1 = half * NQ
                nc.sync.dma_start(out=d_out[h][:, q1 : q1 + 512], in_=outs[:, 0:512])
                nc.sync.dma_start(
                    out=d_out[h][:, q1 + 512 : q1 + NQ], in_=outs[:, 512:NQ]
                )

        for h in range(HPC):
            t_qT = inpool.tile([DA, S], F16, tag="qT")
            t_kT = inpool.tile([DA, S], F16, tag="kT")
            t_v = inpool.tile([128, KB, DA], F16, tag="v")
            if h == 0:
                nc.gpsimd.dma_start(out=t_kT[:, 0:128], in_=d_kT[h][:, 0:128])
                nc.scalar.dma_start(out=t_qT[:, 0:256], in_=d_qT[h][:, 0:256])
                nc.sync.dma_start(out=t_qT[:, 256:512], in_=d_qT[h][:, 256:512])
            else:
                nc.sync.dma_start(out=t_kT[:, 0:128], in_=d_kT[h][:, 0:128])
                nc.sync.dma_start(out=t_qT[:, 0:512], in_=d_qT[h][:, 0:512])
            nc.sync.dma_start(out=t_qT[:, 512:1024], in_=d_qT[h][:, 512:1024])
            nc.sync.dma_start(out=t_kT[:, 128:1024], in_=d_kT[h][:, 128:1024])
            nc.sync.dma_start(out=t_qT[:, 1024:2048], in_=d_qT[h][:, 1024:2048])
            nc.sync.dma_start(out=t_kT[:, 1024:2048], in_=d_kT[h][:, 1024:2048])
            nc.sync.dma_start(out=t_v[:], in_=d_v[h])

            for half in range(2):
                q0 = half * NQ
                t_av = avp.tile([DA, NQ], F32, tag="av", name=f"av_{h}_{half}")
                outs = wkpool.tile([DA, NQ], F32, tag="outT")
                for kb in range(KB):
                    pw = qkp.tile([128, NQ], F32, tag="wave")
                    for j in range(2):
                        nc.tensor.matmul(
                            pw[:, j * 512 : (j + 1) * 512],
                            t_kT[:, kb * 128 : (kb + 1) * 128],
                            t_qT[:, q0 + j * 512 : q0 + (j + 1) * 512],
                            start=True,
                            stop=True,
                        )
                    pt = ptpool.tile([128, NQ], BF16, tag="pt")
                    nc.scalar.activation(
                        pt[:, 0:ACT_W],
                        pw[:, 0:ACT_W],
                        mybir.ActivationFunctionType.Exp,
                        bias=t_bias[:],
                        scale=1.0,
                    )
                    nc.vector.tensor_scalar(
                        pt[:, ACT_W:NQ].bitcast(I16),
                        pw[:, ACT_W:NQ],
                        ALPHA,
                        0.0,
                        op0=mybir.AluOpType.mult,
                        op1=mybir.AluOpType.max,
                    )
                    pend.append((pt, t_v, t_av, kb, outs, h, half))
                    if len(pend) > 3:
                        emit_av(pend.popleft())
                while pend:
                    emit_av(pend.popleft())

    nc.compile()
    return nc


def kernel(
    q: np.ndarray,
    k: np.ndarray,
    v: np.ndarray,
    scale_factor: np.ndarray,
    inv_scale: np.ndarray,
) -> np.ndarray:
    global LAST_RESULT, _CACHED_NC

    q = np.asarray(q, np.float32)
    k = np.asarray(k, np.float32)
    v = np.asarray(v, np.float32)
    scale_factor = np.asarray(scale_factor, np.float32)
    inv_scale = np.asarray(inv_scale, np.float32)

    # host-side input marshaling
    r = 1.0 / (scale_factor * inv_scale[..., None])  # [B,H,S]
    qs = q * r[..., None]  # [B,H,S,D]
    mhat = 5.0 * np.sqrt((qs.astype(np.float64) ** 2).sum(-1)).astype(np.float32)

    # per-head query permutation: most-peaked queries (largest mhat) into the
    # ACT-path column positions of each 1024-chunk
    flat_mhat = mhat.reshape(B * H, S)
    act_pos = np.concatenate(
        [np.arange(0, ACT_W), np.arange(NQ, NQ + ACT_W)]
    )
    dve_pos = np.concatenate(
        [np.arange(ACT_W, NQ), np.arange(NQ + ACT_W, 2 * NQ)]
    )
    pos_list = np.concatenate([act_pos, dve_pos])  # device column for rank j
    perm = np.empty((B * H, S), np.int64)  # device col p holds orig query perm[p]
    srt = np.argsort(-flat_mhat, axis=1, kind="stable")
    for i in range(B * H):
        perm[i, pos_list] = srt[i]
    perm = perm.reshape(B, H, S)

    gb = np.arange(B)[:, None, None]
    gh = np.arange(H)[None, :, None]
    qs_p = qs[gb, gh, perm]  # [B,H,S,D] permuted along queries
    mhat_p = mhat[gb, gh, perm]

    aug = (-mhat_p + np.float32(CBIAS)).astype(np.float32)
    q_aug = np.concatenate([qs_p, aug[..., None]], axis=-1)  # [B,H,S,DA]
    k_aug = np.concatenate([k, np.ones((B, H, S, 1), np.float32)], axis=-1)
    v_aug = np.concatenate([v, np.ones((B, H, S, 1), np.float32)], axis=-1)

    qT = np.ascontiguousarray(q_aug.transpose(0, 1, 3, 2)).astype(np.float16)
    kT = np.ascontiguousarray(k_aug.transpose(0, 1, 3, 2)).astype(np.float16)
    # [B,H,S,DA] -> [B,H,KB,128,DA] -> [B,H,128,KB,DA]
    v16 = np.ascontiguousarray(
        v_aug.reshape(B, H, KB, 128, DA).transpose(0, 1, 3, 2, 4)
    ).astype(np.float16)

    qT = qT.reshape(N_CORES, HPC, DA, S)
    kT = kT.reshape(N_CORES, HPC, DA, S)
    v16 = v16.reshape(N_CORES, HPC, 128, KB, DA)

    _maybe_install_ntff_hook()
    if _CACHED_NC is None:
        _CACHED_NC = _build_nc()
    nc = _CACHED_NC

    in_maps = [{"qT": qT[c], "kT": kT[c], "v": v16[c]} for c in range(N_CORES)]
    res = run_bass_kernel_spmd(nc, in_maps, list(range(N_CORES)))
    LAST_RESULT = res
    outT = np.stack([res.results[c]["outT"] for c in range(N_CORES)])  # [8,HPC,DA,S]
    out = outT[:, :, :D, :] / outT[:, :, D : D + 1, :]
    out = np.ascontiguousarray(out.transpose(0, 1, 3, 2)).reshape(B, H, S, D)
    # inverse query permutation
    unperm = np.empty_like(out)
    unperm[gb, gh, perm] = out
    return unperm.astype(np.float32)
